# revision 1
# baseline (speedup 1.0000x reference)
"""Trainium2 Bass kernel for DinoVisionTransformer Sparse-MoE FC2 (LoRA experts).

Computation (per token t):
    logits = x @ Wg                      -> top-2 softmax-renormalized weights
    out    = x @ W2 + b2 + sum_e cw[t,e] * scale[e] * (x @ A_e) @ B_e

Sharding: data-parallel over the batch dim (8 batch rows -> 8 NeuronCores,
1024 tokens each). All weights replicated.

Per-core kernel (fp16 compute, fp32 PSUM accumulation):
  Phase A (per 128-token tile, contraction over H=4096 in 32 k-chunks of 128):
    stationary = xT tile [128h x 128t]; moving = Wcat[k] columns where
    Wcat = [W2 (1024) | A_flat (512) | Wg_hi (8) | Wg_lo (8)]  (fp16)
    -> psum_base [128,1024], psum_h [128,512], psum_L [128,16]
    Router logits get near-fp32 precision via the split x = x_hi + x_lo:
    logits = x_hi@Wg_hi + x_hi@Wg_lo + x_lo@Wg_hi  (all accumulated in fp32)
  Router (DVE): top-2 of 8 via max8; w1 = sigmoid(l1-l2), w2 = 1-w1;
    dense combine weights cw[t,e] by equality masks. scale[] folded into Bm.
  LoRA: h weighted by cw, transposed 128x128 via PE, then phase B matmuls
    (contraction over E*R=512) accumulate directly into psum_base.
  Final: out = psum_base + b2 (broadcast), DMA to DRAM.
"""

import sys

if "/opt/trn_rl_repo" not in sys.path:
    sys.path.insert(0, "/opt/trn_rl_repo")

import numpy as np

import concourse.bass as bass  # noqa: F401  (registers types)
import concourse.mybir as mybir
import concourse.tile as tile
from concourse import bacc
from concourse.bass import ts
from concourse.bass_utils import run_bass_kernel_spmd
from concourse.masks import make_identity

P = 128
KCH = 32          # H / 128 contraction chunks
TT = 8            # 128-token tiles per core
H = 4096
D = 1024
E = 8
R = 64
ER = E * R        # 512
NW = D + ER + 8 + 8   # 1552 wcat columns
NCORES = 8
WG_K_GROUPS = 8   # wcat DMA split granularity (k-chunks per group)
KPG = KCH // WG_K_GROUPS  # 4

F16 = mybir.dt.float16
F32 = mybir.dt.float32

_CACHE = {}


def _build_nc():
    nc = bacc.Bacc("TRN2")

    xtb_d = nc.dram_tensor("xtb", [TT, P, KCH, P], F16, kind="ExternalInput")
    xlo_d = nc.dram_tensor("xlo", [TT, P, KCH, P], F16, kind="ExternalInput")
    wcat_d = nc.dram_tensor("wcat", [P, KCH, NW], F16, kind="ExternalInput")
    bm_d = nc.dram_tensor("bm", [P, 4, D], F16, kind="ExternalInput")
    b2b_d = nc.dram_tensor("b2b", [P, D], F32, kind="ExternalInput")
    y_d = nc.dram_tensor("y", [TT * P, D], F32, kind="ExternalOutput")

    Sig = mybir.ActivationFunctionType.Sigmoid
    Alu = mybir.AluOpType

    with tile.TileContext(nc) as tc:
        with (
            tc.tile_pool(name="wres", bufs=1) as wres,
            tc.tile_pool(name="xin", bufs=3) as xin,
            tc.tile_pool(name="small", bufs=2) as small,
            tc.tile_pool(name="hbuf", bufs=2) as hbuf,
            tc.tile_pool(name="obuf", bufs=2) as obuf,
            tc.tile_pool(name="ps_base", bufs=2, space="PSUM") as ps_base_pool,
            tc.tile_pool(name="ps_h", bufs=2, space="PSUM") as ps_h_pool,
            tc.tile_pool(name="ps_l", bufs=1, space="PSUM") as ps_l_pool,
            tc.tile_pool(name="ps_t", bufs=1, space="PSUM") as ps_t_pool,
        ):
            # ---- x_hi tiles 0,1 first so phase A can start early; xlo
            # tiles are only needed by the delayed router-correction matmuls,
            # so they issue after the first wcat chunks. HWDGE drains roughly
            # in issue order -> issue in consumption order, no dep chains. ----
            xts = {}
            xlo01 = {}
            for t0 in (0, 1):
                xtb_ = xin.tile([P, KCH, P], F16, tag="xtb")
                xlo_ = xin.tile([P, KCH, P], F16, tag="xlo")
                nc.sync.dma_start(xtb_[:], xtb_d[t0])
                xlo01[t0] = xlo_
                xts[t0] = (xtb_, xlo_)
            wcat_sb = []
            for g in range(WG_K_GROUPS):
                t_ = wres.tile([P, KPG, NW], F16, tag=f"wcat{g}")
                nc.sync.dma_start(t_[:], wcat_d[:, ts(g, KPG), :])
                wcat_sb.append(t_)
                if g == 3:
                    nc.sync.dma_start(xlo01[0][:], xlo_d[0])
                    nc.sync.dma_start(xlo01[1][:], xlo_d[1])
            bm_sb = wres.tile([P, 4, D], F16, tag="bm")
            nc.sync.dma_start(bm_sb[:], bm_d[:])
            b2b_sb = wres.tile([P, D], F32, tag="b2b")
            nc.sync.dma_start(b2b_sb[:], b2b_d[:])
            for t0 in (2, 3):
                xtb_ = xin.tile([P, KCH, P], F16, tag="xtb")
                xlo_ = xin.tile([P, KCH, P], F16, tag="xlo")
                nc.sync.dma_start(xtb_[:], xtb_d[t0])
                nc.sync.dma_start(xlo_[:], xlo_d[t0])
                xts[t0] = (xtb_, xlo_)
            ident = wres.tile([P, P], F16, tag="ident")
            make_identity(nc, ident[:])

            def wc(k, lo, hi):
                return wcat_sb[k // KPG][:, k % KPG, lo:hi]

            # shared logits psum bank: tile t uses half (t % 2).
            # cols [0:16] = x_hi @ [Wg_hi | Wg_lo]; cols [16:24] = 1024x the
            # xlo correction (fp8 operands; rescaled on the DVE afterwards)
            ps_l_shared = ps_l_pool.tile([P, 64], F32, tag="l")

            pend = {}   # t -> (ps_base, ps_h, hwT or None)

            def emit_A_group(t, g, late_xlo=False, warm_only=False):
                """Phase-A matmuls for k-chunks [g*KPG, (g+1)*KPG) of tile t.

                late_xlo: bunch the xlo-correction matmuls into the second
                half of the k-loop (two per slot) so the xlo DMA can be
                issued after the first wcat chunks during startup."""
                xtb_sb, xlo_sb = xts[t]
                if isinstance(xtb_sb, list):
                    def xap(k, _x=xtb_sb):
                        return _x[k // 8][:, k % 8, :]
                else:
                    def xap(k, _x=xtb_sb):
                        return _x[:, k, :]
                ps_base, ps_h, _ = pend[t]
                ps_l = ps_l_shared[:, (t % 2) * 32:(t % 2) * 32 + 32]
                for k in range(g * KPG, (g + 1) * KPG):
                    st = k == 0
                    fin = k == KCH - 1
                    # order: tiny-N matmuls sit between 512-col streams so
                    # their self-loading weight fetches hide under the streams
                    nc.tensor.matmul(
                        ps_base[:, 0:512], xap(k), wc(k, 0, 512),
                        start=st, stop=False, skip_group_check=True,
                    )
                    if not warm_only:
                        nc.tensor.matmul(
                            ps_l[:, 0:16], xap(k), wc(k, 1536, 1552),
                            start=False, stop=False, skip_group_check=True,
                        )
                    nc.tensor.matmul(
                        ps_base[:, 512:1024], xap(k), wc(k, 512, 1024),
                        start=st, stop=False, skip_group_check=True,
                    )
                    nc.tensor.matmul(
                        ps_h[:, :], xap(k), wc(k, 1024, 1536),
                        start=st, stop=fin, skip_group_check=True,
                    )
                    if warm_only:
                        continue
                    if late_xlo:
                        if k >= KCH // 2:
                            for kx in (k - KCH // 2, k):
                                nc.tensor.matmul(
                                    ps_l[:, 0:8], xlo_sb[:, kx, :],
                                    wc(kx, 1536, 1544),
                                    start=False, stop=(kx == KCH - 1),
                                    skip_group_check=True,
                                )
                    else:
                        nc.tensor.matmul(
                            ps_l[:, 0:8], xlo_sb[:, k, :], wc(k, 1536, 1544),
                            start=False, stop=fin, skip_group_check=True,
                        )

            def emit_router_dve(t):
                """Router math + h-weighting (DVE/ACT only); returns hw."""
                ps_base, ps_h, _ = pend[t]
                ps_l = ps_l_shared[:, (t % 2) * 32:(t % 2) * 32 + 32]
                logits = small.tile([P, 8], F32, tag="logits")
                nc.vector.tensor_reduce(
                    logits[:],
                    ps_l[:, 0:16].rearrange("p (s j) -> p j s", s=2),
                    axis=mybir.AxisListType.X,
                    op=Alu.add,
                )
                m8 = small.tile([P, 8], F32, tag="m8")
                nc.vector.max(m8[:], logits[:])
                g_ = small.tile([P, 1], F32, tag="gap")
                nc.vector.tensor_sub(g_[:], m8[:, 0:1], m8[:, 1:2])
                w1 = small.tile([P, 1], F32, tag="w1")
                nc.scalar.activation(w1[:], g_[:], Sig)
                w2 = small.tile([P, 1], F32, tag="w2")
                nc.scalar.activation(w2[:], g_[:], Sig, scale=-1.0)
                cw = small.tile([P, 8], F32, tag="cw")
                cwb = small.tile([P, 8], F32, tag="cwb")
                nc.vector.scalar_tensor_tensor(
                    cw[:], logits[:], m8[:, 0:1], w1[:, 0:1].to_broadcast([P, 8]),
                    op0=Alu.is_equal, op1=Alu.mult,
                )
                nc.vector.scalar_tensor_tensor(
                    cwb[:], logits[:], m8[:, 1:2], w2[:, 0:1].to_broadcast([P, 8]),
                    op0=Alu.is_equal, op1=Alu.mult,
                )
                nc.vector.tensor_add(cw[:], cw[:], cwb[:])
                hw = hbuf.tile([P, ER], F16, tag="hw")
                nc.vector.tensor_tensor(
                    hw.rearrange("p (e r) -> p e r", e=E),
                    ps_h.rearrange("p (e r) -> p e r", e=E),
                    cw[:, :, None].to_broadcast([P, E, R]),
                    Alu.mult,
                )
                return hw

            def emit_router_pe(t, hw):
                """PE transposes of weighted h + copy back; fills pend[t] hwT."""
                ps_base, ps_h, _ = pend[t]
                ps_t = ps_t_pool.tile([P, ER], F16, tag="t")
                for j in range(4):
                    nc.tensor.transpose(
                        ps_t[:, ts(j, P)], hw[:, ts(j, P)], ident[:]
                    )
                hwT = hbuf.tile([P, 4, P], F16, tag="hwT")
                nc.vector.tensor_copy(hwT.rearrange("p a b -> p (a b)"), ps_t[:])
                pend[t] = (ps_base, ps_h, hwT)

            def emit_router(t):
                emit_router_pe(t, emit_router_dve(t))

            def emit_B_and_out(t):
                """LoRA phase B accumulated into base psum, bias add, store."""
                ps_base, _, hwT = pend.pop(t)
                for j in range(4):
                    nc.tensor.matmul(
                        ps_base[:, 0:512], hwT[:, j, :], bm_sb[:, j, 0:512],
                        start=False, stop=False, skip_group_check=True,
                    )
                    nc.tensor.matmul(
                        ps_base[:, 512:1024], hwT[:, j, :], bm_sb[:, j, 512:1024],
                        start=False, stop=(j == 3), skip_group_check=True,
                    )
                out_sb = obuf.tile([P, D], F32, tag="out")
                nc.vector.tensor_add(out_sb[:], ps_base[:], b2b_sb[:])
                nc.scalar.dma_start(y_d[ts(t, P), :], out_sb[:])

            def alloc_psums(t):
                pend[t] = (
                    ps_base_pool.tile([P, D], F32, tag="base", name=f"base{t}"),
                    ps_h_pool.tile([P, ER], F32, tag="h", name=f"h{t}"),
                    None,
                )
                # The shared logits bank must never see start=True (a bank-wide
                # has_written clear would wipe the other tile's half). Instead
                # zero this tile's half; start=False matmuls then accumulate
                # onto 0 (bits set) or overwrite with v (bits clear) — both ok.
                nc.vector.memset(
                    ps_l_shared[:, (t % 2) * 32:(t % 2) * 32 + 32], 0.0
                )

            # ---- startup: interleave phase A of tiles 0 and 1 so the PE has
            # two tiles of work while wcat chunks stream in ----
            D_OFF = 2
            alloc_psums(0)
            alloc_psums(1)
            for g in range(WG_K_GROUPS + D_OFF):
                if g < WG_K_GROUPS:
                    emit_A_group(0, g, late_xlo=True)
                if g == WG_K_GROUPS:
                    emit_router(0)
                gg = g - D_OFF
                if 0 <= gg < WG_K_GROUPS:
                    emit_A_group(1, gg, late_xlo=True)
                if gg == WG_K_GROUPS - 2:
                    emit_B_and_out(0)
            hw_pend = {1: emit_router_dve(1)}

            # ---- steady state ----
            for t in range(2, TT - 1):
                if t >= 4:
                    xtb_ = xin.tile([P, KCH, P], F16, tag="xtb")
                    xlo_ = xin.tile([P, KCH, P], F16, tag="xlo")
                    nc.sync.dma_start(xtb_[:], xtb_d[t])
                    nc.sync.dma_start(xlo_[:], xlo_d[t])
                    xts[t] = (xtb_, xlo_)
                alloc_psums(t)
                for g in range(WG_K_GROUPS):
                    emit_A_group(t, g)
                    if g == 0 and (t - 1) in hw_pend:
                        # previous tile's transposes here: its DVE router
                        # chain is long done, so the PE never stalls on it
                        emit_router_pe(t - 1, hw_pend.pop(t - 1))
                    if g == 4:
                        # previous tile's phase B mid-A so its psum/base slot
                        # frees well before tile t+1 needs it
                        emit_B_and_out(t - 1)
                hw_pend[t] = emit_router_dve(t)

            # ---- last tile: router columns (L, h) stream first so the DVE
            # router chain overlaps the base-column streams; transposes are
            # injected mid-loop -> phase B follows the final matmul directly
            t = TT - 1
            xtb_ = xin.tile([P, KCH, P], F16, tag="xtb")
            xlo_ = xin.tile([P, KCH, P], F16, tag="xlo")
            nc.sync.dma_start(xtb_[:], xtb_d[t])
            nc.sync.dma_start(xlo_[:], xlo_d[t])
            xts[t] = (xtb_, xlo_)
            alloc_psums(t)
            xtb_sb, xlo_sb = xts[t]
            ps_base, ps_h, _ = pend[t]
            ps_l = ps_l_shared[:, (t % 2) * 32:(t % 2) * 32 + 32]
            for k in range(KCH):
                st = k == 0
                fin = k == KCH - 1
                nc.tensor.matmul(
                    ps_l[:, 0:16], xtb_sb[:, k, :], wc(k, 1536, 1552),
                    start=False, stop=False, skip_group_check=True,
                )
                if k == 4 and (t - 1) in hw_pend:
                    emit_router_pe(t - 1, hw_pend.pop(t - 1))
                nc.tensor.matmul(
                    ps_h[:, :], xtb_sb[:, k, :], wc(k, 1024, 1536),
                    start=st, stop=fin, skip_group_check=True,
                )
                nc.tensor.matmul(
                    ps_l[:, 0:8], xlo_sb[:, k, :], wc(k, 1536, 1544),
                    start=False, stop=fin, skip_group_check=True,
                )
            hw_last = emit_router_dve(t)
            for k in range(KCH):
                st = k == 0
                nc.tensor.matmul(
                    ps_base[:, 0:512], xtb_sb[:, k, :], wc(k, 0, 512),
                    start=st, stop=False, skip_group_check=True,
                )
                nc.tensor.matmul(
                    ps_base[:, 512:1024], xtb_sb[:, k, :], wc(k, 512, 1024),
                    start=st, stop=(k == KCH - 1), skip_group_check=True,
                )
                if k == 4:
                    emit_B_and_out(t - 1)
                if k == 10:
                    emit_router_pe(t, hw_last)
            # drain: last tile's phase B with split evac so the first half's
            # bias-add + store overlap the second half's matmuls
            ps_base, _, hwT = pend.pop(TT - 1)
            for j in range(4):
                nc.tensor.matmul(
                    ps_base[:, 0:512], hwT[:, j, :], bm_sb[:, j, 0:512],
                    start=False, stop=(j == 3), skip_group_check=True,
                )
            out_sb = obuf.tile([P, D], F32, tag="out")
            nc.vector.tensor_add(
                out_sb[:, 0:512], ps_base[:, 0:512], b2b_sb[:, 0:512]
            )
            nc.scalar.dma_start(
                y_d[ts(TT - 1, P), 0:512], out_sb[:, 0:512]
            )
            for j in range(4):
                nc.tensor.matmul(
                    ps_base[:, 512:1024], hwT[:, j, :], bm_sb[:, j, 512:1024],
                    start=False, stop=(j == 3), skip_group_check=True,
                )
            nc.vector.tensor_add(
                out_sb[:, 512:1024], ps_base[:, 512:1024], b2b_sb[:, 512:1024]
            )
            nc.scalar.dma_start(
                y_d[ts(TT - 1, P), 512:1024], out_sb[:, 512:1024]
            )

    nc.finalize()
    return nc


def _prep_shared(Wg, W2, b2, A, Bm, scale):
    """Host-side weight layout prep (replicated across cores)."""
    f16, f32 = np.float16, np.float32
    # Wcat = [W2 | A_flat | Wg_hi | Wg_lo], k-chunked to [128, 32, NW]
    a_flat = np.ascontiguousarray(A.transpose(1, 0, 2)).reshape(H, ER)
    wg_hi = Wg.astype(f16)
    wg_lo = (Wg.astype(f32) - wg_hi.astype(f32)).astype(f16)
    wcat = np.empty((H, NW), dtype=f16)
    wcat[:, 0:D] = W2.astype(f16)
    wcat[:, D:D + ER] = a_flat.astype(f16)
    wcat[:, D + ER:D + ER + 8] = wg_hi
    wcat[:, D + ER + 8:] = wg_lo
    wcat = np.ascontiguousarray(wcat.reshape(KCH, P, NW).transpose(1, 0, 2))

    # Bm with scale folded, [(e r), d] -> [128, 4, D]
    bms = (Bm.astype(f32) * scale.astype(f32)[:, None, None]).reshape(ER, D)
    bms = np.ascontiguousarray(bms.reshape(4, P, D).transpose(1, 0, 2)).astype(f16)

    b2b = np.ascontiguousarray(
        np.broadcast_to(b2.astype(f32)[None, :], (P, D))
    )
    return wcat, bms, b2b


def _prep_x_core(x_c):
    """Per-core x prep: fp16 hi + scaled-fp8 lo split, [tile, p, k, ti] layout."""
    f16, f32 = np.float16, np.float32
    xtb = x_c.astype(f16)                                   # [1024, 4096]
    xlo = (x_c.astype(f32) - xtb.astype(f32)).astype(f16)
    def lay(a):
        return np.ascontiguousarray(
            a.reshape(TT, P, KCH, P).transpose(0, 3, 2, 1)
        )
    return lay(xtb), lay(xlo)


def kernel(x, Wg, W2, b2, A, Bm, scale):
    x = np.asarray(x, dtype=np.float32)
    Wg = np.asarray(Wg, dtype=np.float32)
    W2 = np.asarray(W2, dtype=np.float32)
    b2 = np.asarray(b2, dtype=np.float32)
    A = np.asarray(A, dtype=np.float32)
    Bm = np.asarray(Bm, dtype=np.float32)
    scale = np.asarray(scale, dtype=np.float32)

    if "nc" not in _CACHE:
        _CACHE["nc"] = _build_nc()
    nc = _CACHE["nc"]

    wcat, bms, b2b = _prep_shared(Wg, W2, b2, A, Bm, scale)
    in_maps = []
    for c in range(NCORES):
        xtb, xlo = _prep_x_core(x[c])
        in_maps.append(
            {"xtb": xtb, "xlo": xlo, "wcat": wcat, "bm": bms, "b2b": b2b}
        )

    res = run_bass_kernel_spmd(nc, in_maps, core_ids=list(range(NCORES)))
    out = np.stack([res.results[c]["y"] for c in range(NCORES)], axis=0)
    return out.astype(np.float32)



# revision 3
# speedup vs baseline: 1.1457x; 1.1457x over previous
"""Trainium2 Bass kernel for DinoVisionTransformer Sparse-MoE FC2 (LoRA experts).

Computation (per token t):
    logits = x @ Wg                      -> top-2 softmax-renormalized weights
    out    = x @ W2 + b2 + sum_e cw[t,e] * scale[e] * (x @ A_e) @ B_e

Sharding: data-parallel over the batch dim (8 batch rows -> 8 NeuronCores,
1024 tokens each). All weights replicated.

Per-core kernel:
  Base FC2 in fp16 (x fp16 stationary, W2 fp16 moving, fp32 PSUM); W2 is
  pre-scaled by 2^10 so the fp8 LoRA delta can accumulate into the same
  PSUM (see scales below); the final output op divides by 2^10.
  Router logits get near-fp32 precision via the split x = x_hi + x_lo:
  logits = x_hi@Wg_hi + x_hi@Wg_lo + x_lo@Wg_hi (all fp16 operands, fp32
  PSUM). Top-2 of 8 via max8 on the DVE; w1 = sigmoid(l1-l2), w2 = 1-w1;
  dense combine weights cw[t,e] by equality masks.
  LoRA experts run in fp8e4 with DoubleRow perf mode (2 fp8 MACs per PE
  cell per cycle, K=256 per pass -> 2x throughput):
    phase A: h = x8 @ A8 over 16 pairs of 128-k-chunks into fp32 PSUM
    weighting: hw8 = e4m3(h * cw * 2^-6)  (DVE, one scalar_tensor_tensor)
    transpose: 4x PE transpose of hw8 (fp8)
    phase B: delta = hw8T @ Bm8 (DoubleRow) accumulated into the base PSUM
  fp8 scale bookkeeping (all powers of 2, exact):
    x8 = e4m3(x*2^4)   A8 = e4m3(A*2^6)   -> ps_h = h * 2^10
    hw8 = ps_h * cw * 2^-6 = (h*cw) * 2^4
    Bm8 = e4m3(Bm*scale_e*2^6)            -> phaseB psum = delta * 2^10
    W2' = W2 * 2^10 (fp16)                -> ps_base = base * 2^10
    out = ps_base * 2^-10 + b2            (one DVE scalar_tensor_tensor)
"""

import sys

if "/opt/trn_rl_repo" not in sys.path:
    sys.path.insert(0, "/opt/trn_rl_repo")

import ml_dtypes
import numpy as np

import concourse.bass as bass  # noqa: F401  (registers types)
import concourse.mybir as mybir
import concourse.tile as tile
from concourse import bacc
from concourse.bass import ts
from concourse.bass_utils import run_bass_kernel_spmd
from concourse.masks import make_identity

P = 128
KCH = 32          # H / 128 contraction chunks
NPAIR = 16        # KCH / 2 DoubleRow pairs
TT = 8            # 128-token tiles per core
H = 4096
D = 1024
E = 8
R = 64
ER = E * R        # 512
NW = D + 8 + 8    # 1040 wcat columns: [W2*2^10 | Wg_hi | Wg_lo]
NCORES = 8
WG_K_GROUPS = 8   # wcat DMA split granularity (k-chunks per group)
KPG = KCH // WG_K_GROUPS  # 4

F16 = mybir.dt.float16
F32 = mybir.dt.float32
F8 = mybir.dt.float8e4
DR = mybir.MatmulPerfMode.DoubleRow

_CACHE = {}


def _build_nc():
    nc = bacc.Bacc("TRN2")

    xtb_d = nc.dram_tensor("xtb", [TT, P, KCH, P], F16, kind="ExternalInput")
    xlo_d = nc.dram_tensor("xlo", [TT, P, KCH, P], F16, kind="ExternalInput")
    x8_d = nc.dram_tensor("x8", [TT, P, NPAIR, 2, P], F8, kind="ExternalInput")
    wcat_d = nc.dram_tensor("wcat", [P, KCH, NW], F16, kind="ExternalInput")
    a8_d = nc.dram_tensor("a8", [P, NPAIR, 2, ER], F8, kind="ExternalInput")
    bm8_d = nc.dram_tensor("bm8", [P, 4, D], F8, kind="ExternalInput")
    b2b_d = nc.dram_tensor("b2b", [P, D], F32, kind="ExternalInput")
    y_d = nc.dram_tensor("y", [TT * P, D], F32, kind="ExternalOutput")

    Sig = mybir.ActivationFunctionType.Sigmoid
    Alu = mybir.AluOpType

    with tile.TileContext(nc) as tc:
        with (
            tc.tile_pool(name="wres", bufs=1) as wres,
            tc.tile_pool(name="xin", bufs=3) as xin,
            tc.tile_pool(name="small", bufs=2) as small,
            tc.tile_pool(name="hbuf", bufs=2) as hbuf,
            tc.tile_pool(name="obuf", bufs=2) as obuf,
            tc.tile_pool(name="ps_base", bufs=2, space="PSUM") as ps_base_pool,
            tc.tile_pool(name="ps_h", bufs=2, space="PSUM") as ps_h_pool,
            tc.tile_pool(name="ps_l", bufs=1, space="PSUM") as ps_l_pool,
            tc.tile_pool(name="ps_t", bufs=1, space="PSUM") as ps_t_pool,
        ):
            # ---- x tiles 0,1 first so phase A can start early; a8 chunks
            # interleave with the first wcat groups (pair p is consumed in
            # k-group p//2); xlo tiles are only needed by the delayed
            # router-correction matmuls, so they issue after the first wcat
            # chunks. HWDGE drains roughly in issue order -> issue in
            # consumption order. ----
            xts = {}
            xlo01 = {}
            for t0 in (0, 1):
                xtb_ = xin.tile([P, KCH, P], F16, tag="xtb")
                xlo_ = xin.tile([P, KCH, P], F16, tag="xlo")
                x8_ = xin.tile([P, NPAIR, 2, P], F8, tag="x8")
                nc.sync.dma_start(xtb_[:], xtb_d[t0])
                xlo01[t0] = xlo_
                xts[t0] = (xtb_, xlo_, x8_)
            wcat_sb = []
            a8_sb = wres.tile([P, NPAIR, 2, ER], F8, tag="a8")
            for g in range(WG_K_GROUPS):
                t_ = wres.tile([P, KPG, NW], F16, tag=f"wcat{g}")
                nc.sync.dma_start(t_[:], wcat_d[:, ts(g, KPG), :])
                wcat_sb.append(t_)
                if g == 0:
                    # pairs 0..3 (k-groups 0..1), then the x8 stationaries
                    nc.sync.dma_start(a8_sb[:, 0:4], a8_d[:, 0:4])
                    nc.sync.dma_start(xts[0][2][:], x8_d[0])
                    nc.sync.dma_start(xts[1][2][:], x8_d[1])
                if g == 2:
                    nc.sync.dma_start(a8_sb[:, 4:8], a8_d[:, 4:8])
                if g == 3:
                    nc.sync.dma_start(xlo01[0][:], xlo_d[0])
                    nc.sync.dma_start(xlo01[1][:], xlo_d[1])
                if g == 4:
                    nc.sync.dma_start(a8_sb[:, 8:16], a8_d[:, 8:16])
            bm8_sb = wres.tile([P, 4, D], F8, tag="bm8")
            nc.sync.dma_start(bm8_sb[:], bm8_d[:])
            b2b_sb = wres.tile([P, D], F32, tag="b2b")
            nc.sync.dma_start(b2b_sb[:], b2b_d[:])
            for t0 in (2, 3):
                xtb_ = xin.tile([P, KCH, P], F16, tag="xtb")
                xlo_ = xin.tile([P, KCH, P], F16, tag="xlo")
                x8_ = xin.tile([P, NPAIR, 2, P], F8, tag="x8")
                nc.sync.dma_start(xtb_[:], xtb_d[t0])
                nc.sync.dma_start(xlo_[:], xlo_d[t0])
                nc.sync.dma_start(x8_[:], x8_d[t0])
                xts[t0] = (xtb_, xlo_, x8_)
            ident = wres.tile([P, P], F16, tag="ident")
            make_identity(nc, ident[:])

            def wc(k, lo, hi):
                return wcat_sb[k // KPG][:, k % KPG, lo:hi]

            # shared logits psum bank: tile t uses half (t % 2).
            # cols [0:16] = x_hi @ [Wg_hi | Wg_lo]; x_lo @ Wg_hi accumulates
            # onto cols [0:8].
            ps_l_shared = ps_l_pool.tile([P, 64], F32, tag="l")

            pend = {}   # t -> (ps_base, ps_h, hwT or None)

            def emit_A_group(t, g, late_xlo=False):
                """Phase-A matmuls for k-chunks [g*KPG, (g+1)*KPG) of tile t.

                Per even k also emits the fp8 DoubleRow LoRA pass p=k//2
                (contraction chunks k, k+1 of h = x8 @ A8).

                late_xlo: bunch the xlo-correction matmuls into the second
                half of the k-loop (two per slot) so the xlo DMA can be
                issued after the first wcat chunks during startup."""
                xtb_sb, xlo_sb, x8_sb = xts[t]
                ps_base, ps_h, _ = pend[t]
                ps_l = ps_l_shared[:, (t % 2) * 32:(t % 2) * 32 + 32]
                for k in range(g * KPG, (g + 1) * KPG):
                    st = k == 0
                    fin = k == KCH - 1
                    # order: tiny-N matmuls sit between 512-col streams so
                    # their self-loading weight fetches hide under the streams
                    nc.tensor.matmul(
                        ps_base[:, 0:512], xtb_sb[:, k, :], wc(k, 0, 512),
                        start=st, stop=False, skip_group_check=True,
                    )
                    nc.tensor.matmul(
                        ps_l[:, 0:16], xtb_sb[:, k, :], wc(k, D, D + 16),
                        start=False, stop=False, skip_group_check=True,
                    )
                    nc.tensor.matmul(
                        ps_base[:, 512:1024], xtb_sb[:, k, :], wc(k, 512, 1024),
                        start=st, stop=(k == KCH - 1), skip_group_check=True,
                    )
                    if k % 2 == 0:
                        p = k // 2
                        nc.tensor.matmul(
                            ps_h[:, :], x8_sb[:, p, :, :], a8_sb[:, p, :, :],
                            start=(p == 0), stop=(p == NPAIR - 1),
                            perf_mode=DR, skip_group_check=True,
                        )
                    if late_xlo:
                        if k >= KCH // 2:
                            for kx in (k - KCH // 2, k):
                                nc.tensor.matmul(
                                    ps_l[:, 0:8], xlo_sb[:, kx, :],
                                    wc(kx, D, D + 8),
                                    start=False, stop=(kx == KCH - 1),
                                    skip_group_check=True,
                                )
                    else:
                        nc.tensor.matmul(
                            ps_l[:, 0:8], xlo_sb[:, k, :], wc(k, D, D + 8),
                            start=False, stop=fin, skip_group_check=True,
                        )

            def emit_router_dve(t):
                """Router math + h-weighting (DVE/ACT only); returns hw8."""
                ps_base, ps_h, _ = pend[t]
                ps_l = ps_l_shared[:, (t % 2) * 32:(t % 2) * 32 + 32]
                logits = small.tile([P, 8], F32, tag="logits")
                nc.vector.tensor_reduce(
                    logits[:],
                    ps_l[:, 0:16].rearrange("p (s j) -> p j s", s=2),
                    axis=mybir.AxisListType.X,
                    op=Alu.add,
                )
                m8 = small.tile([P, 8], F32, tag="m8")
                nc.vector.max(m8[:], logits[:])
                g_ = small.tile([P, 1], F32, tag="gap")
                nc.vector.tensor_sub(g_[:], m8[:, 0:1], m8[:, 1:2])
                w1 = small.tile([P, 1], F32, tag="w1")
                nc.scalar.activation(w1[:], g_[:], Sig)
                w2 = small.tile([P, 1], F32, tag="w2")
                nc.scalar.activation(w2[:], g_[:], Sig, scale=-1.0)
                cw = small.tile([P, 8], F32, tag="cw")
                cwb = small.tile([P, 8], F32, tag="cwb")
                nc.vector.scalar_tensor_tensor(
                    cw[:], logits[:], m8[:, 0:1], w1[:, 0:1].to_broadcast([P, 8]),
                    op0=Alu.is_equal, op1=Alu.mult,
                )
                nc.vector.scalar_tensor_tensor(
                    cwb[:], logits[:], m8[:, 1:2], w2[:, 0:1].to_broadcast([P, 8]),
                    op0=Alu.is_equal, op1=Alu.mult,
                )
                nc.vector.tensor_add(cw[:], cw[:], cwb[:])
                # hw16 = ps_h * 2^-6 * cw = (h*cw) * 2^4  (e4m3 grid after
                # the post-transpose copy converts to fp8)
                hw16 = hbuf.tile([P, ER], F16, tag="hw16")
                nc.vector.scalar_tensor_tensor(
                    hw16.rearrange("p (e r) -> p e r", e=E),
                    ps_h.rearrange("p (e r) -> p e r", e=E),
                    2.0 ** -6,
                    cw[:, :, None].to_broadcast([P, E, R]),
                    op0=Alu.mult, op1=Alu.mult,
                )
                return hw16

            def emit_router_pe(t, hw16):
                """PE transposes of weighted h (fp16) + fp8-converting copy."""
                ps_base, ps_h, _ = pend[t]
                ps_t = ps_t_pool.tile([P, ER], F16, tag="t")
                for j in range(4):
                    nc.tensor.transpose(
                        ps_t[:, ts(j, P)], hw16[:, ts(j, P)], ident[:]
                    )
                hwT = hbuf.tile([P, 4, P], F8, tag="hwT")
                nc.vector.tensor_copy(hwT.rearrange("p a b -> p (a b)"), ps_t[:])
                pend[t] = (ps_base, ps_h, hwT)

            def emit_router(t):
                emit_router_pe(t, emit_router_dve(t))

            def emit_B_and_out(t):
                """LoRA phase B (fp8 DoubleRow) into base psum, bias, store."""
                ps_base, _, hwT = pend.pop(t)
                for c in (0, 2):
                    for dh in (0, 512):
                        nc.tensor.matmul(
                            ps_base[:, dh:dh + 512],
                            hwT[:, c:c + 2, :], bm8_sb[:, c:c + 2, dh:dh + 512],
                            start=False, stop=(c == 2),
                            perf_mode=DR, skip_group_check=True,
                        )
                out_sb = obuf.tile([P, D], F32, tag="out")
                nc.vector.scalar_tensor_tensor(
                    out_sb[:], ps_base[:], 2.0 ** -10, b2b_sb[:],
                    op0=Alu.mult, op1=Alu.add,
                )
                nc.scalar.dma_start(y_d[ts(t, P), :], out_sb[:])

            def alloc_psums(t):
                pend[t] = (
                    ps_base_pool.tile([P, D], F32, tag="base", name=f"base{t}"),
                    ps_h_pool.tile([P, ER], F32, tag="h", name=f"h{t}"),
                    None,
                )
                # The shared logits bank must never see start=True (a bank-wide
                # has_written clear would wipe the other tile's half). Instead
                # zero this tile's half; start=False matmuls then accumulate
                # onto 0 (bits set) or overwrite with v (bits clear) — both ok.
                nc.vector.memset(
                    ps_l_shared[:, (t % 2) * 32:(t % 2) * 32 + 32], 0.0
                )

            # ---- startup: interleave phase A of tiles 0 and 1 so the PE has
            # two tiles of work while wcat chunks stream in ----
            D_OFF = 2
            alloc_psums(0)
            alloc_psums(1)
            for g in range(WG_K_GROUPS + D_OFF):
                if g < WG_K_GROUPS:
                    emit_A_group(0, g, late_xlo=True)
                if g == WG_K_GROUPS:
                    emit_router(0)
                gg = g - D_OFF
                if 0 <= gg < WG_K_GROUPS:
                    emit_A_group(1, gg, late_xlo=True)
                if gg == WG_K_GROUPS - 2:
                    emit_B_and_out(0)
            hw_pend = {1: emit_router_dve(1)}

            # ---- steady state ----
            for t in range(2, TT - 1):
                if t >= 4:
                    xtb_ = xin.tile([P, KCH, P], F16, tag="xtb")
                    xlo_ = xin.tile([P, KCH, P], F16, tag="xlo")
                    x8_ = xin.tile([P, NPAIR, 2, P], F8, tag="x8")
                    nc.sync.dma_start(xtb_[:], xtb_d[t])
                    nc.sync.dma_start(xlo_[:], xlo_d[t])
                    nc.sync.dma_start(x8_[:], x8_d[t])
                    xts[t] = (xtb_, xlo_, x8_)
                alloc_psums(t)
                for g in range(WG_K_GROUPS):
                    emit_A_group(t, g)
                    if g == 0 and (t - 1) in hw_pend:
                        # previous tile's transposes here: its DVE router
                        # chain is long done, so the PE never stalls on it
                        emit_router_pe(t - 1, hw_pend.pop(t - 1))
                    if g == 4:
                        # previous tile's phase B mid-A so its psum/base slot
                        # frees well before tile t+1 needs it
                        emit_B_and_out(t - 1)
                hw_pend[t] = emit_router_dve(t)

            # ---- last tile: router columns (L, h) stream first so the DVE
            # router chain overlaps the base-column streams; transposes are
            # injected mid-loop -> phase B follows the final matmul directly
            t = TT - 1
            xtb_ = xin.tile([P, KCH, P], F16, tag="xtb")
            xlo_ = xin.tile([P, KCH, P], F16, tag="xlo")
            x8_ = xin.tile([P, NPAIR, 2, P], F8, tag="x8")
            nc.sync.dma_start(xtb_[:], xtb_d[t])
            nc.sync.dma_start(xlo_[:], xlo_d[t])
            nc.sync.dma_start(x8_[:], x8_d[t])
            xts[t] = (xtb_, xlo_, x8_)
            alloc_psums(t)
            xtb_sb, xlo_sb, x8_sb = xts[t]
            ps_base, ps_h, _ = pend[t]
            ps_l = ps_l_shared[:, (t % 2) * 32:(t % 2) * 32 + 32]
            for k in range(KCH):
                st = k == 0
                fin = k == KCH - 1
                nc.tensor.matmul(
                    ps_l[:, 0:16], xtb_sb[:, k, :], wc(k, D, D + 16),
                    start=False, stop=False, skip_group_check=True,
                )
                if k == 4 and (t - 1) in hw_pend:
                    emit_router_pe(t - 1, hw_pend.pop(t - 1))
                if k % 2 == 0:
                    p = k // 2
                    nc.tensor.matmul(
                        ps_h[:, :], x8_sb[:, p, :, :], a8_sb[:, p, :, :],
                        start=(p == 0), stop=(p == NPAIR - 1),
                        perf_mode=DR, skip_group_check=True,
                    )
                nc.tensor.matmul(
                    ps_l[:, 0:8], xlo_sb[:, k, :], wc(k, D, D + 8),
                    start=False, stop=fin, skip_group_check=True,
                )
            hw_last = emit_router_dve(t)
            for k in range(KCH):
                st = k == 0
                nc.tensor.matmul(
                    ps_base[:, 0:512], xtb_sb[:, k, :], wc(k, 0, 512),
                    start=st, stop=False, skip_group_check=True,
                )
                nc.tensor.matmul(
                    ps_base[:, 512:1024], xtb_sb[:, k, :], wc(k, 512, 1024),
                    start=st, stop=(k == KCH - 1), skip_group_check=True,
                )
                if k == 4:
                    emit_B_and_out(t - 1)
                if k == 10:
                    emit_router_pe(t, hw_last)
            # drain: last tile's phase B with split evac so the first half's
            # bias-add + store overlap the second half's matmuls
            ps_base, _, hwT = pend.pop(TT - 1)
            for c in (0, 2):
                nc.tensor.matmul(
                    ps_base[:, 0:512], hwT[:, c:c + 2, :],
                    bm8_sb[:, c:c + 2, 0:512],
                    start=False, stop=(c == 2),
                    perf_mode=DR, skip_group_check=True,
                )
            out_sb = obuf.tile([P, D], F32, tag="out")
            nc.vector.scalar_tensor_tensor(
                out_sb[:, 0:512], ps_base[:, 0:512], 2.0 ** -10,
                b2b_sb[:, 0:512], op0=Alu.mult, op1=Alu.add,
            )
            nc.scalar.dma_start(
                y_d[ts(TT - 1, P), 0:512], out_sb[:, 0:512]
            )
            for c in (0, 2):
                nc.tensor.matmul(
                    ps_base[:, 512:1024], hwT[:, c:c + 2, :],
                    bm8_sb[:, c:c + 2, 512:1024],
                    start=False, stop=(c == 2),
                    perf_mode=DR, skip_group_check=True,
                )
            nc.vector.scalar_tensor_tensor(
                out_sb[:, 512:1024], ps_base[:, 512:1024], 2.0 ** -10,
                b2b_sb[:, 512:1024], op0=Alu.mult, op1=Alu.add,
            )
            nc.scalar.dma_start(
                y_d[ts(TT - 1, P), 512:1024], out_sb[:, 512:1024]
            )

    nc.finalize()
    return nc


F8NP = ml_dtypes.float8_e4m3fn


def _prep_shared(Wg, W2, b2, A, Bm, scale):
    """Host-side weight layout prep (replicated across cores)."""
    f16, f32 = np.float16, np.float32
    # Wcat = [W2*2^10 | Wg_hi | Wg_lo], k-chunked to [128, 32, NW]
    wg_hi = Wg.astype(f16)
    wg_lo = (Wg.astype(f32) - wg_hi.astype(f32)).astype(f16)
    wcat = np.empty((H, NW), dtype=f16)
    wcat[:, 0:D] = (W2.astype(f32) * 1024.0).astype(f16)
    wcat[:, D:D + 8] = wg_hi
    wcat[:, D + 8:] = wg_lo
    wcat = np.ascontiguousarray(wcat.reshape(KCH, P, NW).transpose(1, 0, 2))

    # A8: e4m3(A * 2^6), [e,h,r] -> flat [h, (e r)] -> DoubleRow pairs
    # [128h, pair, slot, er]
    a_flat = np.ascontiguousarray(A.transpose(1, 0, 2)).reshape(H, ER)
    a8 = (a_flat.astype(f32) * 64.0).astype(F8NP)
    a8 = np.ascontiguousarray(
        a8.reshape(NPAIR, 2, P, ER).transpose(2, 0, 1, 3)
    )

    # Bm8 = e4m3(Bm * scale_e * 2^6), [(e r), d] -> [128er, 4chunk, D]
    bms = (Bm.astype(f32) * scale.astype(f32)[:, None, None] * 64.0).reshape(ER, D)
    bm8 = np.ascontiguousarray(bms.reshape(4, P, D).transpose(1, 0, 2)).astype(F8NP)

    b2b = np.ascontiguousarray(
        np.broadcast_to(b2.astype(f32)[None, :], (P, D))
    )
    return wcat, a8, bm8, b2b


def _prep_x_core(x_c):
    """Per-core x prep: fp16 hi + fp16 lo split plus e4m3 x8 for LoRA.

    xtb/xlo: [tile, 128h, k, 128t];  x8: [tile, 128h, pair, slot, 128t]."""
    f16, f32 = np.float16, np.float32
    xtb = x_c.astype(f16)                                   # [1024, 4096]
    xlo = (x_c.astype(f32) - xtb.astype(f32)).astype(f16)
    def lay(a):
        return np.ascontiguousarray(
            a.reshape(TT, P, KCH, P).transpose(0, 3, 2, 1)
        )
    x8 = (x_c.astype(f32) * 16.0).astype(F8NP)
    x8 = np.ascontiguousarray(
        x8.reshape(TT, P, NPAIR, 2, P).transpose(0, 4, 2, 3, 1)
    )
    return lay(xtb), lay(xlo), x8


def build_in_maps(x, Wg, W2, b2, A, Bm, scale):
    wcat, a8, bm8, b2b = _prep_shared(Wg, W2, b2, A, Bm, scale)
    in_maps = []
    for c in range(NCORES):
        xtb, xlo, x8 = _prep_x_core(x[c])
        in_maps.append(
            {"xtb": xtb, "xlo": xlo, "x8": x8, "wcat": wcat,
             "a8": a8, "bm8": bm8, "b2b": b2b}
        )
    return in_maps


def kernel(x, Wg, W2, b2, A, Bm, scale):
    x = np.asarray(x, dtype=np.float32)
    Wg = np.asarray(Wg, dtype=np.float32)
    W2 = np.asarray(W2, dtype=np.float32)
    b2 = np.asarray(b2, dtype=np.float32)
    A = np.asarray(A, dtype=np.float32)
    Bm = np.asarray(Bm, dtype=np.float32)
    scale = np.asarray(scale, dtype=np.float32)

    if "nc" not in _CACHE:
        _CACHE["nc"] = _build_nc()
    nc = _CACHE["nc"]

    in_maps = build_in_maps(x, Wg, W2, b2, A, Bm, scale)
    res = run_bass_kernel_spmd(nc, in_maps, core_ids=list(range(NCORES)))
    out = np.stack([res.results[c]["y"] for c in range(NCORES)], axis=0)
    return out.astype(np.float32)
